# revision 36
# baseline (speedup 1.0000x reference)
"""Causal multi-head attention (B=2, L=2048, D=2048, NH=16, HD=128) on 8
Trainium2 NeuronCores.

Sharding: core c = b*4 + g handles batch b and head-group g (4 heads).
Each core computes q/k/v projections for its 512 features, causal
attention for its 4 heads, and the partial o-projection
attn_out @ Wo[:, g_cols].T -> [L, D].  The host sums the 4 per-batch
partials and adds bo.

Structure (weights resident per phase to minimize HBM traffic):
  phase 1a: q,k projections for the whole sequence; wq+wk resident,
            x streamed in 256-token half-blocks.
  phase 1b: v projection; wv resident, x streamed again.
  phase 2:  flash-style causal attention (scores computed directly in
            the transposed [k, q] layout; softmax without max-shift —
            scores ~ N(0,1); row sums via ones-matmul; normalization by
            broadcast reciprocal) fused with the partial o-projection;
            wo resident.

All data is fp32 bits; matmul operands are typed float32r, which the PE
runs at full rate when the moving free dim >= 256 (~4x fp32 throughput,
~1e-3 component relative error).
"""

import sys

for _p in ("/opt/trn_rl_repo",):
    if _p not in sys.path:
        sys.path.insert(0, _p)

import numpy as np
from contextlib import ExitStack

import concourse.bass as bass  # noqa: F401
import concourse.tile as tile
from concourse import bacc, mybir
from concourse import bass_utils

P = 128
B, L, D = 2, 2048, 2048
NH, HD = 16, 128
SCALE = HD ** -0.5
G = 8 // B            # head-groups per batch = 4
H = 4                 # heads per core
F = H * HD            # 512 features per core
TB = 512              # token block (q-block)
NTB = L // TB         # 4
HB = 256              # half-block for projection streaming (moving dim)
KT = D // P           # 16 contraction tiles for projections

f32r = mybir.dt.float32r
f32 = mybir.dt.float32

_CACHE = {}


def _build(reps=1):
    key = ("nc", reps)
    if key in _CACHE:
        return _CACHE[key]

    nc = bacc.Bacc("TRN2", target_bir_lowering=False, debug=False, num_devices=8)

    xT = nc.dram_tensor("xT", [D, L], f32r, kind="ExternalInput").ap()
    wq = nc.dram_tensor("wq", [P, H, KT, HD], f32r, kind="ExternalInput").ap()
    wk = nc.dram_tensor("wk", [P, H, KT, HD], f32r, kind="ExternalInput").ap()
    wv = nc.dram_tensor("wv", [P, KT, F], f32r, kind="ExternalInput").ap()
    wo = nc.dram_tensor("wo", [P, H, D], f32r, kind="ExternalInput").ap()
    bqv = nc.dram_tensor("bqv", [F], f32r, kind="ExternalInput").ap()
    bkv = nc.dram_tensor("bkv", [F], f32r, kind="ExternalInput").ap()
    bvv = nc.dram_tensor("bvv", [F], f32r, kind="ExternalInput").ap()
    ones = nc.dram_tensor("ones", [TB], f32r, kind="ExternalInput").ap()
    o = nc.dram_tensor("o", [L, D], f32, kind="ExternalOutput").ap()

    xT3 = xT.rearrange("(kt p) t -> p kt t", p=P)

    with tile.TileContext(nc) as tc:
        with ExitStack() as ctx:
            ctx.enter_context(nc.allow_low_precision(reason="f32r is 4-byte fp32"))
            consts = ctx.enter_context(tc.tile_pool(name="consts", bufs=1))
            resid = ctx.enter_context(tc.tile_pool(name="resid", bufs=1))

            # ---- constants ----
            # triangular additive mask [P, P] in [k, q] orientation:
            # keep (0.0) where k_local <= q_local, else -1e30
            tri = consts.tile([P, P], f32, name="tri")
            nc.gpsimd.memset(tri[:], 0.0)
            nc.gpsimd.affine_select(
                out=tri[:],
                in_=tri[:],
                compare_op=mybir.AluOpType.is_ge,
                fill=-1e30,
                base=0,
                pattern=[[1, P]],
                channel_multiplier=-1,
            )

            ones_col = consts.tile([P, 1], f32r)
            nc.scalar.dma_start(ones_col[:], ones[:P, None])
            ones_row = consts.tile([1, TB], f32r)
            nc.scalar.dma_start(ones_row[:], ones[None, :])
            bv_row = consts.tile([1, F], f32r)
            nc.scalar.dma_start(bv_row[:], bvv[None, :])
            # per-partition bias tiles for the q/k copies: [P, H]
            bq_pp = consts.tile([P, H], f32, name="bq_pp")
            nc.scalar.dma_start(bq_pp[:], bqv.rearrange("(h p) -> p h", p=P).bitcast(f32))
            bk_pp = consts.tile([P, H], f32, name="bk_pp")
            nc.scalar.dma_start(bk_pp[:], bkv.rearrange("(h p) -> p h", p=P).bitcast(f32))

            # ---- persistent activations ----
            qT_sb = resid.tile([P, H, L], f32r, name="qT_sb")     # 4 MiB
            kT_sb = resid.tile([P, H, L], f32r, name="kT_sb")     # 4 MiB
            v_sb = resid.tile([P, L // P, F], f32r, name="v_sb")  # 4 MiB

            rep_ctx = ExitStack()
            if reps > 1:
                # timing mode: repeat the whole body in a hardware loop
                rep_ctx.enter_context(tc.For_i(0, reps, 1))

            # ================= phase 1a: q,k projections =================
            with ExitStack() as p1:
                wres_pool = p1.enter_context(tc.tile_pool(name="wres", bufs=1))
                xpool = p1.enter_context(tc.tile_pool(name="xpool", bufs=2))
                psA = p1.enter_context(tc.tile_pool(name="psA", bufs=4, space="PSUM"))

                wq_res = wres_pool.tile([P, H, KT, HD], f32r, name="wq_res")
                wk_res = wres_pool.tile([P, H, KT, HD], f32r, name="wk_res")
                # interleave per-head loads so the first groups start early
                nc.sync.dma_start(wq_res[:, 0], wq[:, 0])
                x_first = [None]

                for half in range(2 * NTB):
                    xt = xpool.tile([P, KT, HB], f32r, tag="xT")
                    nc.sync.dma_start(xt[:], xT3[:, :, half * HB : (half + 1) * HB])
                    if half == 0:
                        # remaining weight loads, finest-first for pipelining
                        nc.sync.dma_start(wk_res[:, 0], wk[:, 0])
                        for hh in range(1, H):
                            nc.sync.dma_start(wq_res[:, hh], wq[:, hh])
                            nc.sync.dma_start(wk_res[:, hh], wk[:, hh])
                    for wres, bias_pp, dst in (
                        (wq_res, bq_pp, qT_sb),
                        (wk_res, bk_pp, kT_sb),
                    ):
                        for h in range(H):
                            ps = psA.tile([P, HB], f32, tag="psA")
                            for kt in range(KT):
                                nc.tensor.matmul(
                                    ps[:],
                                    wres[:, h, kt],
                                    xt[:, kt],
                                    start=(kt == 0),
                                    stop=(kt == KT - 1),
                                )
                            nc.scalar.activation(
                                dst[:, h, half * HB : (half + 1) * HB],
                                ps[:],
                                mybir.ActivationFunctionType.Identity,
                                bias=bias_pp[:, h : h + 1],
                                scale=1.0,
                            )

            # ================= phase 2: attention + o-projection ==========
            with ExitStack() as p2:
                wo_pool = p2.enter_context(tc.tile_pool(name="wop", bufs=1))
                wv_pool = p2.enter_context(tc.tile_pool(name="wvres", bufs=1))
                xq_pool = p2.enter_context(tc.tile_pool(name="xq", bufs=2))
                apool = p2.enter_context(tc.tile_pool(name="apool", bufs=1))
                ptpool = p2.enter_context(tc.tile_pool(name="ptpool", bufs=2))
                spool = p2.enter_context(tc.tile_pool(name="spool", bufs=1))
                ostg = p2.enter_context(tc.tile_pool(name="ostg", bufs=3))
                psS = p2.enter_context(tc.tile_pool(name="psS", bufs=3, space="PSUM"))
                psPO = p2.enter_context(tc.tile_pool(name="psPO", bufs=2, space="PSUM"))
                psR = p2.enter_context(tc.tile_pool(name="psR", bufs=1, space="PSUM"))
                psC = p2.enter_context(tc.tile_pool(name="psC", bufs=2, space="PSUM"))

                wo_res = wo_pool.tile([P, H, D], f32r, name="wo_res")
                wo_loaded = [False]
                wv_res = wv_pool.tile([P, KT, F], f32r, name="wv_res")
                FH = F // 2
                nc.sync.dma_start(wv_res[:, :, :FH], wv[:, :, :FH])
                wv_fh1_loaded = [False]

                def emit_v(tb):
                    # v projection for the 4 token-tiles of block tb, split
                    # by feature half (first half needs only 2 MiB of wv);
                    # x quarter is the stationary operand
                    for q4 in range(TB // P):
                        xq = xq_pool.tile([P, KT, P], f32r, tag="xq")
                        lo = tb * TB + q4 * P
                        nc.sync.dma_start(xq[:], xT3[:, :, lo : lo + P])
                        if not wv_fh1_loaded[0]:
                            nc.sync.dma_start(wv_res[:, :, FH:], wv[:, :, FH:])
                            wv_fh1_loaded[0] = True
                        for fh in range(2):
                            fsl = slice(fh * FH, (fh + 1) * FH)
                            ps = psC.tile([P, F], f32, tag="psC")
                            for kt in range(KT):
                                nc.tensor.matmul(
                                    ps[:, :FH],
                                    xq[:, kt],
                                    wv_res[:, kt, fsl],
                                    start=(kt == 0),
                                    stop=False,
                                )
                            # bias: rank-1  ones (x) bv
                            nc.tensor.matmul(
                                ps[:, :FH],
                                ones_row[:, :P],
                                bv_row[:, fsl],
                                start=False,
                                stop=True,
                            )
                            nc.scalar.copy(v_sb[:, tb * (TB // P) + q4, fsl], ps[:, :FH])

                pts = {}
                po_h = {}
                rsum_h = {}
                att_tb = {}

                def emit_score(tb, h, jt):
                    s = psS.tile([P, TB], f32, tag="s")
                    nc.tensor.matmul(
                        s[:],
                        kT_sb[:, h, jt * P : (jt + 1) * P],
                        qT_sb[:, h, tb * TB : (tb + 1) * TB],
                        start=True,
                        stop=True,
                    )
                    jl = jt - 4 * tb
                    if jl >= 0:
                        # left of the diagonal subtile: fully masked columns
                        if jl > 0:
                            nc.vector.tensor_scalar_add(
                                s[:, : jl * P], s[:, : jl * P], -1e30
                            )
                        # diagonal 128-col subtile: triangular mask
                        nc.vector.tensor_tensor(
                            s[:, jl * P : (jl + 1) * P],
                            s[:, jl * P : (jl + 1) * P],
                            tri[:],
                            mybir.AluOpType.add,
                        )
                    pt = ptpool.tile([P, TB], f32r, tag="pt")
                    nc.scalar.activation(
                        pt[:], s[:], mybir.ActivationFunctionType.Exp
                    )
                    pts[(tb, h, jt)] = pt

                def emit_rp(tb, h, jt):
                    njt = 4 * (tb + 1)
                    pt = pts.pop((tb, h, jt))
                    if jt == 0:
                        po_h[(tb, h)] = psPO.tile([P, TB], f32, tag="po", name=f"po{tb}_{h}")
                        rsum_h[(tb, h)] = psR.tile([1, TB], f32, tag="rsum", name=f"rs{tb}_{h}")
                    nc.tensor.matmul(
                        rsum_h[(tb, h)][:], ones_col[:], pt[:],
                        start=(jt == 0), stop=(jt == njt - 1),
                    )
                    nc.tensor.matmul(
                        po_h[(tb, h)][:], v_sb[:, jt, h * HD : (h + 1) * HD], pt[:],
                        start=(jt == 0), stop=(jt == njt - 1),
                    )
                    if jt == njt - 1:
                        emit_tail(tb, h)

                def emit_tail(tb, h):
                    po = po_h.pop((tb, h))
                    rsum = rsum_h.pop((tb, h))
                    recip = spool.tile([1, TB], f32r, tag="recip")
                    nc.vector.reciprocal(recip[:], rsum[:])
                    bc_ps = psS.tile([P, TB], f32, tag="s")
                    nc.tensor.matmul(
                        bc_ps[:], ones_row[:, :P], recip[:], start=True, stop=True
                    )
                    bc = spool.tile([P, TB], f32, tag="bc")
                    nc.scalar.copy(bc[:], bc_ps[:])
                    nc.vector.tensor_tensor(
                        att_tb[tb][:, h, :], po[:], bc[:], mybir.AluOpType.mult
                    )

                def emit_oproj(tb):
                    att_sb = att_tb.pop(tb)
                    for ob in range(D // TB):
                        for tt in range(TB // P):
                            ps = psC.tile([P, TB], f32, tag="psC")
                            for h in range(H):
                                nc.tensor.matmul(
                                    ps[:],
                                    att_sb[:, h, tt * P : (tt + 1) * P],
                                    wo_res[:, h, ob * TB : (ob + 1) * TB],
                                    start=(h == 0),
                                    stop=(h == H - 1),
                                )
                            ot = ostg.tile([P, TB], f32, tag="ostg")
                            nc.scalar.copy(ot[:], ps[:])
                            nc.sync.dma_start(
                                o[
                                    tb * TB + tt * P : tb * TB + (tt + 1) * P,
                                    ob * TB : (ob + 1) * TB,
                                ],
                                ot[:],
                            )

                # one flat software pipeline across (tb, head, j-tile): the
                # score matmul leads the rsum/PV step by one so the PE never
                # waits on ACT's exp; the o-projection for block tb is
                # emitted right after its last head completes.
                tasks = []
                for tb in range(NTB):
                    for h in range(H):
                        for jt in range(4 * (tb + 1)):
                            tasks.append((tb, h, jt))

                for tb in range(NTB):
                    att_tb[tb] = apool.tile([P, H, TB], f32r, tag="att", name=f"att{tb}")

                emit_v(0)
                emit_score(*tasks[0])
                for i in range(1, len(tasks)):
                    emit_score(*tasks[i])
                    emit_rp(*tasks[i - 1])
                    if i == 8 and not wo_loaded[0]:
                        # defer the 4 MiB wo load so it does not delay the
                        # first score matmuls at the phase boundary
                        nc.sync.dma_start(wo_res[:], wo[:])
                        wo_loaded[0] = True
                    tb_prev = tasks[i - 1][0]
                    if tasks[i][0] != tb_prev:
                        emit_v(tb_prev + 1)
                        emit_oproj(tb_prev)
                emit_rp(*tasks[-1])
                emit_oproj(NTB - 1)

            rep_ctx.close()

    nc.compile()
    _CACHE[key] = nc
    return nc


def _in_maps(hidden_states, Wq, bq, Wk, bk, Wv, bv, Wo, bo):
    hs = np.asarray(hidden_states, np.float32)
    Wq = np.asarray(Wq, np.float32)
    Wk = np.asarray(Wk, np.float32)
    Wv = np.asarray(Wv, np.float32)
    Wo = np.asarray(Wo, np.float32)
    bq = np.asarray(bq, np.float32)
    bk = np.asarray(bk, np.float32)
    bv = np.asarray(bv, np.float32)

    ones = np.ones((TB,), np.float32)
    maps = []
    for b in range(B):
        xT = np.ascontiguousarray(hs[b].T)
        for g in range(G):
            sl = slice(g * F, (g + 1) * F)
            wqT = (Wq[sl, :].T * SCALE).astype(np.float32)   # (D, F)
            wkT = Wk[sl, :].T                                 # (D, F)
            wvT = Wv[sl, :].T                                 # (D, F)
            woT = Wo[:, sl].T                                 # (F, D)
            maps.append(
                {
                    "xT": xT,
                    "wq": np.ascontiguousarray(
                        wqT.reshape(KT, P, H, HD).transpose(1, 2, 0, 3)
                    ),
                    "wk": np.ascontiguousarray(
                        wkT.reshape(KT, P, H, HD).transpose(1, 2, 0, 3)
                    ),
                    "wv": np.ascontiguousarray(
                        wvT.reshape(KT, P, F).transpose(1, 0, 2)
                    ),
                    "wo": np.ascontiguousarray(
                        woT.reshape(H, HD, D).transpose(1, 0, 2)
                    ),
                    "bqv": np.ascontiguousarray(bq[sl] * SCALE),
                    "bkv": np.ascontiguousarray(bk[sl]),
                    "bvv": np.ascontiguousarray(bv[sl]),
                    "ones": ones,
                }
            )
    return maps


def kernel(hidden_states, Wq, bq, Wk, bk, Wv, bv, Wo, bo, **run_kwargs):
    nc = _build()
    maps = _in_maps(hidden_states, Wq, bq, Wk, bk, Wv, bv, Wo, bo)
    res = bass_utils.run_bass_kernel_spmd(
        nc, maps, core_ids=list(range(8)), **run_kwargs
    )
    bo = np.asarray(bo, np.float32)
    out = np.empty((B, L, D), np.float32)
    for b in range(B):
        acc = res.results[b * G]["o"].astype(np.float32).copy()
        for g in range(1, G):
            acc += res.results[b * G + g]["o"]
        out[b] = acc + bo[None, :]
    _CACHE["last_res"] = res
    return out


# revision 44
# speedup vs baseline: 1.1054x; 1.1054x over previous
"""Causal multi-head attention (B=2, L=2048, D=2048, NH=16, HD=128) on 8
Trainium2 NeuronCores.

Sharding: core c = b*4 + g handles batch b and head-group g (4 heads).
Each core computes q/k/v projections for its 512 features, causal
attention for its 4 heads, and the partial o-projection
attn_out @ Wo[:, g_cols].T -> [L, D].  The host sums the 4 per-batch
partials and adds bo.

Structure (weights resident per phase to minimize HBM traffic):
  phase 1a: q,k projections for the whole sequence; wq+wk resident,
            x streamed in 256-token half-blocks.
  phase 1b: v projection; wv resident, x streamed again.
  phase 2:  flash-style causal attention (scores computed directly in
            the transposed [k, q] layout; softmax without max-shift —
            scores ~ N(0,1); row sums via ones-matmul; normalization by
            broadcast reciprocal) fused with the partial o-projection;
            wo resident.

All data is fp32 bits; matmul operands are typed float32r, which the PE
runs at full rate when the moving free dim >= 256 (~4x fp32 throughput,
~1e-3 component relative error).
"""

import sys

for _p in ("/opt/trn_rl_repo",):
    if _p not in sys.path:
        sys.path.insert(0, _p)

import numpy as np
from contextlib import ExitStack

import concourse.bass as bass  # noqa: F401
import concourse.tile as tile
from concourse import bacc, mybir
from concourse import bass_utils

P = 128
B, L, D = 2, 2048, 2048
NH, HD = 16, 128
SCALE = HD ** -0.5
G = 8 // B            # head-groups per batch = 4
H = 4                 # heads per core
F = H * HD            # 512 features per core
TB = 512              # token block (q-block)
NTB = L // TB         # 4
HB = 256              # half-block for projection streaming (moving dim)
KT = D // P           # 16 contraction tiles for projections

f32r = mybir.dt.float32r
f32 = mybir.dt.float32

_CACHE = {}


def _build(reps=1):
    key = ("nc", reps)
    if key in _CACHE:
        return _CACHE[key]

    nc = bacc.Bacc("TRN2", target_bir_lowering=False, debug=False, num_devices=8)

    xT = nc.dram_tensor("xT", [D, L], f32r, kind="ExternalInput").ap()
    wq = nc.dram_tensor("wq", [P, H, KT, HD], f32r, kind="ExternalInput").ap()
    wk = nc.dram_tensor("wk", [P, H, KT, HD], f32r, kind="ExternalInput").ap()
    wv = nc.dram_tensor("wv", [P, KT, F], f32r, kind="ExternalInput").ap()
    wo = nc.dram_tensor("wo", [P, H, D], f32r, kind="ExternalInput").ap()
    bqv = nc.dram_tensor("bqv", [F], f32r, kind="ExternalInput").ap()
    bkv = nc.dram_tensor("bkv", [F], f32r, kind="ExternalInput").ap()
    bvv = nc.dram_tensor("bvv", [F], f32r, kind="ExternalInput").ap()
    ones = nc.dram_tensor("ones", [TB], f32r, kind="ExternalInput").ap()
    o = nc.dram_tensor("o", [L, D], f32, kind="ExternalOutput").ap()

    xT3 = xT.rearrange("(kt p) t -> p kt t", p=P)

    with tile.TileContext(nc) as tc:
        with ExitStack() as ctx:
            ctx.enter_context(nc.allow_low_precision(reason="f32r is 4-byte fp32"))
            consts = ctx.enter_context(tc.tile_pool(name="consts", bufs=1))
            resid = ctx.enter_context(tc.tile_pool(name="resid", bufs=1))

            # ---- constants ----
            # triangular additive mask [P, P] in [k, q] orientation:
            # keep (0.0) where k_local <= q_local, else -1e30
            tri = consts.tile([P, P], f32, name="tri")
            nc.gpsimd.memset(tri[:], 0.0)
            nc.gpsimd.affine_select(
                out=tri[:],
                in_=tri[:],
                compare_op=mybir.AluOpType.is_ge,
                fill=-1e30,
                base=0,
                pattern=[[1, P]],
                channel_multiplier=-1,
            )

            ones_col = consts.tile([P, 1], f32r)
            nc.scalar.dma_start(ones_col[:], ones[:P, None])
            ones_row = consts.tile([1, TB], f32r)
            nc.scalar.dma_start(ones_row[:], ones[None, :])
            bv_row = consts.tile([1, F], f32r)
            nc.scalar.dma_start(bv_row[:], bvv[None, :])
            # per-partition bias tiles for the q/k copies: [P, H]
            bq_pp = consts.tile([P, H], f32, name="bq_pp")
            nc.scalar.dma_start(bq_pp[:], bqv.rearrange("(h p) -> p h", p=P).bitcast(f32))
            bk_pp = consts.tile([P, H], f32, name="bk_pp")
            nc.scalar.dma_start(bk_pp[:], bkv.rearrange("(h p) -> p h", p=P).bitcast(f32))

            # ---- persistent activations ----
            qT_sb = resid.tile([P, H, L], f32r, name="qT_sb")     # 4 MiB
            kT_sb = resid.tile([P, H, L], f32r, name="kT_sb")     # 4 MiB

            rep_ctx = ExitStack()
            if reps > 1:
                # timing mode: repeat the whole body in a hardware loop
                rep_ctx.enter_context(tc.For_i(0, reps, 1))

            # ================= phase 1a: q,k projections =================
            with ExitStack() as p1:
                wres_pool = p1.enter_context(tc.tile_pool(name="wres", bufs=1))
                xpool = p1.enter_context(tc.tile_pool(name="xpool", bufs=2))
                psA = p1.enter_context(tc.tile_pool(name="psA", bufs=4, space="PSUM"))

                wq_res = wres_pool.tile([P, H, KT, HD], f32r, name="wq_res")
                wk_res = wres_pool.tile([P, H, KT, HD], f32r, name="wk_res")
                # interleave per-head loads so the first groups start early
                nc.sync.dma_start(wq_res[:, 0], wq[:, 0])
                x_first = [None]

                # chunks: two 256-halves for block 0 (early start while the
                # weights stream in), then full 512 blocks (half the
                # instruction count)
                chunks = [(0, HB), (HB, HB)] + [
                    (tb * TB, TB) for tb in range(1, NTB)
                ]
                for ci, (lo, ln) in enumerate(chunks):
                    xt = xpool.tile([P, KT, TB], f32r, tag="xT")
                    nc.sync.dma_start(xt[:, :, :ln], xT3[:, :, lo : lo + ln])
                    if ci == 0:
                        # remaining weight loads, finest-first for pipelining
                        nc.sync.dma_start(wk_res[:, 0], wk[:, 0])
                        for hh in range(1, H):
                            nc.sync.dma_start(wq_res[:, hh], wq[:, hh])
                            nc.sync.dma_start(wk_res[:, hh], wk[:, hh])
                    for wres, bias_pp, dst in (
                        (wq_res, bq_pp, qT_sb),
                        (wk_res, bk_pp, kT_sb),
                    ):
                        for h in range(H):
                            ps = psA.tile([P, TB], f32, tag="psA")
                            for kt in range(KT):
                                nc.tensor.matmul(
                                    ps[:, :ln],
                                    wres[:, h, kt],
                                    xt[:, kt, :ln],
                                    start=(kt == 0),
                                    stop=(kt == KT - 1),
                                )
                            nc.scalar.activation(
                                dst[:, h, lo : lo + ln],
                                ps[:, :ln],
                                mybir.ActivationFunctionType.Identity,
                                bias=bias_pp[:, h : h + 1],
                                scale=1.0,
                            )

            # ================= phase 2: attention + o-projection ==========
            with ExitStack() as p2:
                wo_pool = p2.enter_context(tc.tile_pool(name="wop", bufs=1))
                wv_pool = p2.enter_context(tc.tile_pool(name="wvres", bufs=1))
                xq_pool = p2.enter_context(tc.tile_pool(name="xq", bufs=2))
                apool = p2.enter_context(tc.tile_pool(name="apool", bufs=1))
                ptpool = p2.enter_context(tc.tile_pool(name="ptpool", bufs=3))
                spool = p2.enter_context(tc.tile_pool(name="spool", bufs=1))
                ostg = p2.enter_context(tc.tile_pool(name="ostg", bufs=3))
                psS = p2.enter_context(tc.tile_pool(name="psS", bufs=3, space="PSUM"))
                psPO = p2.enter_context(tc.tile_pool(name="psPO", bufs=2, space="PSUM"))
                psR = p2.enter_context(tc.tile_pool(name="psR", bufs=1, space="PSUM"))
                psC = p2.enter_context(tc.tile_pool(name="psC", bufs=2, space="PSUM"))

                vpool = p2.enter_context(tc.tile_pool(name="vpool", bufs=1))
                v_sb = vpool.tile([P, L // P, F], f32r, name="v_sb")  # 4 MiB
                wo_res = wo_pool.tile([P, H, D], f32r, name="wo_res")
                wo_loaded = [False]
                wv_res = wv_pool.tile([P, KT, F], f32r, name="wv_res")
                FH = F // 2
                nc.sync.dma_start(wv_res[:, :, :FH], wv[:, :, :FH])
                wv_fh1_loaded = [False]

                def emit_v(tb):
                    # v projection for the 4 token-tiles of block tb, split
                    # by feature half (first half needs only 2 MiB of wv);
                    # x quarter is the stationary operand
                    for q4 in range(TB // P):
                        xq = xq_pool.tile([P, KT, P], f32r, tag="xq")
                        lo = tb * TB + q4 * P
                        nc.sync.dma_start(xq[:], xT3[:, :, lo : lo + P])
                        if not wv_fh1_loaded[0]:
                            nc.sync.dma_start(wv_res[:, :, FH:], wv[:, :, FH:])
                            wv_fh1_loaded[0] = True
                        # feature-split only matters for block 0 (first PVs
                        # unblock after half of wv); full width elsewhere
                        fsls = (
                            [slice(0, FH), slice(FH, F)] if tb == 0 else [slice(0, F)]
                        )
                        for fsl in fsls:
                            fn_ = fsl.stop - fsl.start
                            ps = psC.tile([P, F], f32, tag="psC")
                            for kt in range(KT):
                                nc.tensor.matmul(
                                    ps[:, :fn_],
                                    xq[:, kt],
                                    wv_res[:, kt, fsl],
                                    start=(kt == 0),
                                    stop=False,
                                )
                            # bias: rank-1  ones (x) bv
                            nc.tensor.matmul(
                                ps[:, :fn_],
                                ones_row[:, :P],
                                bv_row[:, fsl],
                                start=False,
                                stop=True,
                            )
                            nc.scalar.copy(v_sb[:, tb * (TB // P) + q4, fsl], ps[:, :fn_])

                pts = {}
                po_h = {}
                rsum_h = {}
                att_tb = {}

                def emit_score(tb, h, jt):
                    s = psS.tile([P, TB], f32, tag="s")
                    nc.tensor.matmul(
                        s[:],
                        kT_sb[:, h, jt * P : (jt + 1) * P],
                        qT_sb[:, h, tb * TB : (tb + 1) * TB],
                        start=True,
                        stop=True,
                    )
                    jl = jt - 4 * tb
                    if jl >= 0:
                        # left of the diagonal subtile: fully masked columns
                        if jl > 0:
                            nc.vector.tensor_scalar_add(
                                s[:, : jl * P], s[:, : jl * P], -1e30
                            )
                        # diagonal 128-col subtile: triangular mask
                        nc.vector.tensor_tensor(
                            s[:, jl * P : (jl + 1) * P],
                            s[:, jl * P : (jl + 1) * P],
                            tri[:],
                            mybir.AluOpType.add,
                        )
                    pt = ptpool.tile([P, TB], f32r, tag="pt")
                    nc.scalar.activation(
                        pt[:], s[:], mybir.ActivationFunctionType.Exp
                    )
                    pts[(tb, h, jt)] = pt

                def emit_rp(tb, h, jt):
                    njt = 4 * (tb + 1)
                    pt = pts.pop((tb, h, jt))
                    if jt == 0:
                        po_h[(tb, h)] = psPO.tile([P, TB], f32, tag="po", name=f"po{tb}_{h}")
                        rsum_h[(tb, h)] = psR.tile([1, TB], f32, tag="rsum", name=f"rs{tb}_{h}")
                    nc.tensor.matmul(
                        rsum_h[(tb, h)][:], ones_col[:], pt[:],
                        start=(jt == 0), stop=(jt == njt - 1),
                    )
                    nc.tensor.matmul(
                        po_h[(tb, h)][:], v_sb[:, jt, h * HD : (h + 1) * HD], pt[:],
                        start=(jt == 0), stop=(jt == njt - 1),
                    )
                    if jt == njt - 1:
                        emit_tail(tb, h)

                def emit_tail(tb, h):
                    po = po_h.pop((tb, h))
                    rsum = rsum_h.pop((tb, h))
                    recip = spool.tile([1, TB], f32r, tag="recip")
                    nc.vector.reciprocal(recip[:], rsum[:])
                    bc_ps = psS.tile([P, TB], f32, tag="s")
                    nc.tensor.matmul(
                        bc_ps[:], ones_row[:, :P], recip[:], start=True, stop=True
                    )
                    bc = spool.tile([P, TB], f32, tag="bc")
                    nc.scalar.copy(bc[:], bc_ps[:])
                    nc.vector.tensor_tensor(
                        att_tb[tb][:, h, :], po[:], bc[:], mybir.AluOpType.mult
                    )

                def emit_oproj(tb):
                    att_sb = att_tb.pop(tb)
                    for ob in range(D // TB):
                        for tt in range(TB // P):
                            ps = psC.tile([P, TB], f32, tag="psC")
                            for h in range(H):
                                nc.tensor.matmul(
                                    ps[:],
                                    att_sb[:, h, tt * P : (tt + 1) * P],
                                    wo_res[:, h, ob * TB : (ob + 1) * TB],
                                    start=(h == 0),
                                    stop=(h == H - 1),
                                )
                            ot = ostg.tile([P, TB], f32, tag="ostg")
                            nc.scalar.copy(ot[:], ps[:])
                            nc.sync.dma_start(
                                o[
                                    tb * TB + tt * P : tb * TB + (tt + 1) * P,
                                    ob * TB : (ob + 1) * TB,
                                ],
                                ot[:],
                            )

                # one flat software pipeline across (tb, head, j-tile): the
                # score matmul leads the rsum/PV step by one so the PE never
                # waits on ACT's exp; the o-projection for block tb is
                # emitted right after its last head completes.
                tasks = []
                for tb in range(NTB):
                    for h in range(H):
                        for jt in range(4 * (tb + 1)):
                            tasks.append((tb, h, jt))

                for tb in range(NTB):
                    att_tb[tb] = apool.tile([P, H, TB], f32r, tag="att", name=f"att{tb}")

                emit_v(0)
                emit_score(*tasks[0])
                for i in range(1, len(tasks)):
                    emit_score(*tasks[i])
                    emit_rp(*tasks[i - 1])
                    if i == 8 and not wo_loaded[0]:
                        # defer the 4 MiB wo load so it does not delay the
                        # first score matmuls at the phase boundary
                        nc.sync.dma_start(wo_res[:], wo[:])
                        wo_loaded[0] = True
                    tb_prev = tasks[i - 1][0]
                    if tasks[i][0] != tb_prev:
                        emit_v(tb_prev + 1)
                        emit_oproj(tb_prev)
                emit_rp(*tasks[-1])
                emit_oproj(NTB - 1)

            rep_ctx.close()

    nc.compile()
    _CACHE[key] = nc
    return nc


def _in_maps(hidden_states, Wq, bq, Wk, bk, Wv, bv, Wo, bo):
    hs = np.asarray(hidden_states, np.float32)
    Wq = np.asarray(Wq, np.float32)
    Wk = np.asarray(Wk, np.float32)
    Wv = np.asarray(Wv, np.float32)
    Wo = np.asarray(Wo, np.float32)
    bq = np.asarray(bq, np.float32)
    bk = np.asarray(bk, np.float32)
    bv = np.asarray(bv, np.float32)

    ones = np.ones((TB,), np.float32)
    maps = []
    for b in range(B):
        xT = np.ascontiguousarray(hs[b].T)
        for g in range(G):
            sl = slice(g * F, (g + 1) * F)
            wqT = (Wq[sl, :].T * SCALE).astype(np.float32)   # (D, F)
            wkT = Wk[sl, :].T                                 # (D, F)
            wvT = Wv[sl, :].T                                 # (D, F)
            woT = Wo[:, sl].T                                 # (F, D)
            maps.append(
                {
                    "xT": xT,
                    "wq": np.ascontiguousarray(
                        wqT.reshape(KT, P, H, HD).transpose(1, 2, 0, 3)
                    ),
                    "wk": np.ascontiguousarray(
                        wkT.reshape(KT, P, H, HD).transpose(1, 2, 0, 3)
                    ),
                    "wv": np.ascontiguousarray(
                        wvT.reshape(KT, P, F).transpose(1, 0, 2)
                    ),
                    "wo": np.ascontiguousarray(
                        woT.reshape(H, HD, D).transpose(1, 0, 2)
                    ),
                    "bqv": np.ascontiguousarray(bq[sl] * SCALE),
                    "bkv": np.ascontiguousarray(bk[sl]),
                    "bvv": np.ascontiguousarray(bv[sl]),
                    "ones": ones,
                }
            )
    return maps


def kernel(hidden_states, Wq, bq, Wk, bk, Wv, bv, Wo, bo, **run_kwargs):
    nc = _build()
    maps = _in_maps(hidden_states, Wq, bq, Wk, bk, Wv, bv, Wo, bo)
    res = bass_utils.run_bass_kernel_spmd(
        nc, maps, core_ids=list(range(8)), **run_kwargs
    )
    bo = np.asarray(bo, np.float32)
    out = np.empty((B, L, D), np.float32)
    for b in range(B):
        acc = res.results[b * G]["o"].astype(np.float32).copy()
        for g in range(1, G):
            acc += res.results[b * G + g]["o"]
        out[b] = acc + bo[None, :]
    _CACHE["last_res"] = res
    return out
